# revision 1
# baseline (speedup 1.0000x reference)
"""MHA with KV cache on 8 trn2 NeuronCores — v4: wide scores psum + per-batch o_proj overlap.

Problem (hardcoded): B=2, S=1024, HIDDEN=2048, HEADS=16, HEAD_DIM=128,
PAST=1024, KV=2048.  out = softmax(mask(q k^T / sqrt(d))) v -> o_proj.
Sharding: 2 heads per core, row-parallel o_proj, host sums 8 partials.

Differences vs the v1 baseline:
- Scores are computed TRANSPOSED ([kv partitions, q free]) so softmax'd
  probs feed the PV matmul directly — no per-tile PE transposes and no
  DVE repack of probs.
- Softmax denominators come from a ones-vector matmul over the same fp16
  probs; normalization is folded into the PV output ([128, S] per block,
  via reciprocal + gpsimd partition-broadcast + one DVE multiply)
  instead of a full [128, KV] probs pass.
- Everything runs fp16 into fp32 PSUM (fp8 was measured 2x faster on the
  PE but costs >=1.5e-2 relative error here — too close to the 2e-2
  gate).  Inputs stream as fp16, halving DMA vs fp32.
- The causal mask is added by the PE itself (identity x maskT matmul
  into the scores psum) instead of a DVE pass.
"""
import numpy as np

import concourse.bass as bass
import concourse.mybir as mybir
import concourse.tile as tile
from concourse import bacc
from concourse.bass_utils import run_bass_kernel_spmd
from concourse.masks import make_identity

FP32 = mybir.dt.float32
FP16 = mybir.dt.float16
AF = mybir.ActivationFunctionType
ALU = mybir.AluOpType

B, S, HID, HEADS, D, PAST = 2, 1024, 2048, 16, 128, 1024
KV = PAST + S
P = 128
NCORES = 8
HPC = HEADS // NCORES          # heads per core = 2
CD = HPC * D                   # per-core projection dims = 256
TOK = B * S
TCK = 512                      # projection token chunk
KC = HID // P                  # contraction tiles for projections
NEG = -30000.0                 # fp16-representable -inf for the causal mask
CFG = {"mode": "full"}         # full | dma_only | no_dma  (timing ablations)


def _ntiles(b):
    return 16 if b == 0 else 15   # b=1: last 128 kv are padding


def _qlo(c):
    return max(0, (c - 8) * P)    # first valid query col for kv tile c


def build(reps=1, loop_n=None):
    nc = bacc.Bacc()

    x16 = nc.dram_tensor("x16", [P, KC, TOK], FP16, kind="ExternalInput")
    wq = nc.dram_tensor("wq", [P, KC, CD], FP16, kind="ExternalInput")
    wk = nc.dram_tensor("wk", [P, KC, CD], FP16, kind="ExternalInput")
    wv = nc.dram_tensor("wv", [P, KC, CD], FP16, kind="ExternalInput")
    wo = nc.dram_tensor("wo", [P, HPC, HID], FP16, kind="ExternalInput")
    bq = nc.dram_tensor("bq", [CD], FP32, kind="ExternalInput")
    bk = nc.dram_tensor("bk", [CD], FP32, kind="ExternalInput")
    bv = nc.dram_tensor("bv", [CD], FP32, kind="ExternalInput")
    bo = nc.dram_tensor("bo", [HID], FP32, kind="ExternalInput")
    pkt = nc.dram_tensor("pkt", [P, B, HPC, PAST], FP16, kind="ExternalInput")
    pvt = nc.dram_tensor("pvt", [P, B, HPC, PAST // P, D], FP16,
                         kind="ExternalInput")
    maskT = nc.dram_tensor("maskT", [P, P], FP16, kind="ExternalInput")
    outT = nc.dram_tensor("outT", [HID, TOK], FP16, kind="ExternalOutput")

    with tile.TileContext(nc) as tc:
        with (
            tc.tile_pool(name="consts", bufs=1) as consts,
            tc.tile_pool(name="acts", bufs=1) as acts,
        ):
            ident = consts.tile([P, P], FP16)
            make_identity(nc, ident)
            mask_sb = consts.tile([P, P], FP16)
            nc.sync.dma_start(mask_sb[:], maskT[:])
            ones16 = consts.tile([P, 1], FP16)
            nc.any.memset(ones16[:], 1.0)
            bq_sb = consts.tile([P, HPC], FP32)
            nc.sync.dma_start(bq_sb[:], bq.rearrange("(c p) -> p c", p=P))
            bk_sb = consts.tile([P, HPC], FP32)
            nc.sync.dma_start(bk_sb[:], bk.rearrange("(c p) -> p c", p=P))
            bv_sb = consts.tile([P, HPC], FP32)
            nc.sync.dma_start(bv_sb[:], bv.rearrange("(c p) -> p c", p=P))
            bo_sb = consts.tile([P, HID // P], FP32)
            nc.sync.dma_start(bo_sb[:], bo.rearrange("(c p) -> p c", p=P))

            qT = acts.tile([P, B, HPC, S], FP16, tag="qT")
            kT = acts.tile([P, B, HPC, KV], FP16, tag="kT")
            v16 = acts.tile([P, B, HPC, KV // P, D], FP16, tag="v16")
            om16 = acts.tile([P, HPC, B, S], FP16, tag="om16")
            wqs = acts.tile([P, KC, CD], FP16, tag="wqs")
            wks = acts.tile([P, KC, CD], FP16, tag="wks")
            wvs = acts.tile([P, KC, CD], FP16, tag="wvs")
            wos = acts.tile([P, HPC, HID], FP16, tag="wos")

            if CFG["mode"] == "no_dma":
                # init-once garbage so the loop body never reads unwritten
                # tiles (all DMA is ablated away inside the loop)
                for t in (wqs, wks, wvs, wos, kT, v16):
                    nc.any.memset(t[:], 0.001)
                xg = acts.tile([P, KC, TCK], FP16, tag="xg")
                nc.any.memset(xg[:], 0.001)
            else:
                xg = None

            if loop_n is not None:
                env = dict(locals())
                with tc.For_i(0, loop_n, 1):
                    _body(nc, tc, 0, env)
            else:
                for rep in range(reps):
                    _body(nc, tc, rep, locals())

    nc.finalize()
    return nc


def _body(nc, tc, rep, env):
    ident = env["ident"]; mask_sb = env["mask_sb"]; ones16 = env["ones16"]
    bq_sb = env["bq_sb"]; bk_sb = env["bk_sb"]; bv_sb = env["bv_sb"]
    bo_sb = env["bo_sb"]
    qT = env["qT"]; kT = env["kT"]; v16 = env["v16"]; om16 = env["om16"]
    wqs = env["wqs"]; wks = env["wks"]; wvs = env["wvs"]; wos = env["wos"]
    x16 = env["x16"]; wq = env["wq"]; wk = env["wk"]; wv = env["wv"]
    wo = env["wo"]; pkt = env["pkt"]; pvt = env["pvt"]; outT = env["outT"]

    mode = CFG["mode"]
    dma_on = mode != "no_dma"
    compute_on = mode != "dma_only"
    xg = env.get("xg")

    # ---------------- phase 1: q/k/v projections (fp16) -------------------
    if dma_on:
        nc.sync.dma_start(wqs[:], wq[:])
    with (
        tc.tile_pool(name=f"xtp{rep}", bufs=2) as xtp,
        tc.tile_pool(name=f"pps{rep}", bufs=4, space="PSUM") as pps,
        tc.tile_pool(name=f"tvps{rep}", bufs=2, space="PSUM") as tvps,
        tc.tile_pool(name=f"vstg{rep}", bufs=2) as vstg,
    ):
        for t0 in range(0, TOK, TCK):
            b = t0 // S
            s0 = t0 % S
            xc = xg if xg is not None else xtp.tile([P, KC, TCK], FP16,
                                                     tag="xc")
            if dma_on:
                nc.sync.dma_start(xc[:], x16[:, :, t0:t0 + TCK])
                if t0 == 0:
                    nc.sync.dma_start(wks[:], wk[:])
                    nc.sync.dma_start(wvs[:], wv[:])
                    nc.sync.dma_start(kT[:, :, :, 0:PAST], pkt[:])
                    nc.sync.dma_start(v16[:, :, :, 0:PAST // P, :], pvt[:])
                    nc.sync.dma_start(wos[:], wo[:])
            if not compute_on:
                continue
            for w_sb, kind in ((wqs, "q"), (wks, "k"), (wvs, "v")):
                for jb in range(HPC):
                    ps = pps.tile([P, TCK], FP32, tag="pps")
                    for c in range(KC):
                        nc.tensor.matmul(
                            ps[:], w_sb[:, c, jb * P:(jb + 1) * P],
                            xc[:, c, :], start=(c == 0), stop=(c == KC - 1))
                    if kind == "q":
                        nc.vector.tensor_scalar_add(
                            qT[:, b, jb, s0:s0 + TCK], ps[:],
                            bq_sb[:, jb:jb + 1])
                    elif kind == "k":
                        nc.vector.tensor_scalar_add(
                            kT[:, b, jb, PAST + s0:PAST + s0 + TCK], ps[:],
                            bk_sb[:, jb:jb + 1])
                    else:
                        vt16 = vstg.tile([P, TCK], FP16, tag="vt16")
                        nc.scalar.activation(
                            vt16[:], ps[:], AF.Identity,
                            bias=bv_sb[:, jb:jb + 1], scale=1.0)
                        for i in range(TCK // P):
                            kvt = (PAST + s0) // P + i
                            tp = tvps.tile([P, P], FP16, tag="tp")
                            nc.tensor.matmul(tp[:], vt16[:, i * P:(i + 1) * P],
                                             ident[:], is_transpose=True)
                            nc.vector.tensor_copy(v16[:, b, jb, kvt, :], tp[:])

    if not compute_on:
        return _dma_only_out(nc, tc, rep, env)
    # ------- phases 2+3: attention + per-batch o_proj (overlapped) --------
    with (
        tc.tile_pool(name=f"scps{rep}", bufs=2, space="PSUM") as scps,
        tc.tile_pool(name=f"dps{rep}", bufs=1, space="PSUM") as dps,
        tc.tile_pool(name=f"pvps{rep}", bufs=1, space="PSUM") as pvps,
        tc.tile_pool(name=f"probs{rep}", bufs=2) as probs_pool,
        tc.tile_pool(name=f"rec{rep}", bufs=2) as rec,
        tc.tile_pool(name=f"ops{rep}", bufs=2, space="PSUM") as ops,
        tc.tile_pool(name=f"ostg{rep}", bufs=6) as ostg,
    ):
        eng = 0
        for b in range(B):
            for h in range(HPC):
                nt = _ntiles(b)
                pT = probs_pool.tile([P, 16, S], FP16, tag="pT")
                for c in range(nt):
                    qlo = _qlo(c)
                    diag = c >= 8
                    sc = scps.tile([P, S], FP32, tag="sc")
                    for ch0 in (0, 512):
                        lo = max(ch0, qlo)
                        hi = ch0 + 512
                        if lo >= hi:
                            continue
                        in_diag = diag and lo <= qlo < hi
                        nc.tensor.matmul(
                            sc[:, lo:hi], kT[:, b, h, c * P:(c + 1) * P],
                            qT[:, b, h, lo:hi],
                            start=True, stop=not in_diag,
                            skip_group_check=True)
                        if in_diag:
                            nc.tensor.matmul(
                                sc[:, qlo:qlo + P], ident[:], mask_sb[:],
                                start=False, stop=True, skip_group_check=True)
                    nc.scalar.activation(pT[:, c, qlo:S], sc[:, qlo:S], AF.Exp)
                for qh in (0, 512):
                    # denominators + PV for this q-half
                    dn = dps.tile([1, 512], FP32, tag="dn")
                    pv = pvps.tile([P, 512], FP32, tag="pv")
                    cs = [c for c in range(nt) if _qlo(c) < qh + 512]
                    for c in cs:
                        lo = max(qh, _qlo(c))
                        o = lo - qh
                        first, last = c == cs[0], c == cs[-1]
                        nc.tensor.matmul(
                            dn[0:1, o:512], ones16[:], pT[:, c, lo:qh + 512],
                            start=first, stop=last, skip_group_check=True)
                        nc.tensor.matmul(
                            pv[:, o:512], v16[:, b, h, c, :],
                            pT[:, c, lo:qh + 512],
                            start=first, stop=last, skip_group_check=True)
                    r0 = rec.tile([1, 512], FP32, tag="r0")
                    nc.vector.reciprocal(r0[:], dn[:])
                    recB = rec.tile([P, 512], FP32, tag="recB")
                    nc.gpsimd.partition_broadcast(recB[:], r0[:], channels=P)
                    nc.vector.tensor_tensor(
                        om16[:, h, b, qh:qh + 512], pv[:], recB[:],
                        op=ALU.mult)
            # ---- o_proj partial for this batch (overlaps next batch) -----
            for s0 in range(0, S, TCK):
                for mb in range(HID // P):
                    op = ops.tile([P, TCK], FP32, tag="op")
                    for j in range(HPC):
                        nc.tensor.matmul(
                            op[:], wos[:, j, mb * P:(mb + 1) * P],
                            om16[:, j, b, s0:s0 + TCK],
                            start=(j == 0), stop=(j == HPC - 1))
                    stg = ostg.tile([P, TCK], FP16, tag="stg")
                    if eng == 0:
                        nc.vector.tensor_scalar_add(
                            stg[:], op[:], bo_sb[:, mb:mb + 1])
                    else:
                        nc.scalar.activation(
                            stg[:], op[:], AF.Identity,
                            bias=bo_sb[:, mb:mb + 1], scale=1.0)
                    eng = (eng + 1) % 2
                    if dma_on:
                        nc.sync.dma_start(
                            outT[mb * P:(mb + 1) * P,
                                 b * S + s0:b * S + s0 + TCK],
                            stg[:])


def _dma_only_out(nc, tc, rep, env):
    outT = env["outT"]
    with tc.tile_pool(name=f"ostg{rep}", bufs=2) as ostg:
        for b in range(B):
            for s0 in range(0, S, TCK):
                for mb in range(HID // P):
                    stg = ostg.tile([P, TCK], FP16, tag="stg")
                    nc.any.memset(stg[0:1, 0:1], 0.0)
                    nc.sync.dma_start(
                        outT[mb * P:(mb + 1) * P, b * S + s0:b * S + s0 + TCK],
                        stg[:])


_cached_nc = None


def _get_nc():
    global _cached_nc
    if _cached_nc is None:
        _cached_nc = build()
    return _cached_nc


def _prep_in_maps(inputs):
    X = np.asarray(inputs["X"], dtype=np.float32)
    past_k = np.asarray(inputs["past_k"], dtype=np.float32)
    past_v = np.asarray(inputs["past_v"], dtype=np.float32)
    Wq = np.asarray(inputs["Wq"], dtype=np.float32)
    Wk = np.asarray(inputs["Wk"], dtype=np.float32)
    Wv = np.asarray(inputs["Wv"], dtype=np.float32)
    Wo = np.asarray(inputs["Wo"], dtype=np.float32)
    bq = np.asarray(inputs["bq"], dtype=np.float32)
    bk = np.asarray(inputs["bk"], dtype=np.float32)
    bv = np.asarray(inputs["bv"], dtype=np.float32)
    bo = np.asarray(inputs["bo"], dtype=np.float32)

    scale = np.float32(1.0 / np.sqrt(D))
    # X^T packed [p, c, tok], hid = c*128 + p
    xt = X.reshape(TOK, HID).T.reshape(KC, P, TOK).transpose(1, 0, 2)
    x16 = np.ascontiguousarray(xt).astype(np.float16)
    maskT = np.tril(np.full((P, P), NEG, dtype=np.float32), k=-1).astype(
        np.float16)

    def packw(Wslice):
        # [256 feat, HID] -> [p, c, feat]
        wt = Wslice.T.reshape(KC, P, CD).transpose(1, 0, 2)
        return np.ascontiguousarray(wt).astype(np.float16)

    in_maps = []
    for cidx in range(NCORES):
        lo, hi = cidx * CD, (cidx + 1) * CD
        # wo: [HID, 256] slice -> [p(dh within head), j(head), HID]
        wo_t = Wo[:, lo:hi].T.reshape(HPC, P, HID).transpose(1, 0, 2)
        pkt = np.ascontiguousarray(
            past_k[:, cidx * HPC:(cidx + 1) * HPC].transpose(3, 0, 1, 2)
        ).astype(np.float16)
        # past_v: [b,h,kv,d] -> [p(kv within tile), b, h, tile, d]
        pv = past_v[:, cidx * HPC:(cidx + 1) * HPC].reshape(
            B, HPC, PAST // P, P, D).transpose(3, 0, 1, 2, 4)
        in_maps.append({
            "x16": x16,
            "wq": packw(Wq[lo:hi] * scale),
            "wk": packw(Wk[lo:hi]),
            "wv": packw(Wv[lo:hi]),
            "wo": np.ascontiguousarray(wo_t).astype(np.float16),
            "bq": np.ascontiguousarray(bq[lo:hi] * scale),
            "bk": np.ascontiguousarray(bk[lo:hi]),
            "bv": np.ascontiguousarray(bv[lo:hi]),
            "bo": bo if cidx == 0 else np.zeros_like(bo),
            "pkt": pkt,
            "pvt": np.ascontiguousarray(pv).astype(np.float16),
            "maskT": maskT,
        })
    return in_maps


def _run(inputs, trace=False, nc=None):
    if nc is None:
        nc = _get_nc()
    in_maps = _prep_in_maps(inputs)
    res = run_bass_kernel_spmd(nc, in_maps, core_ids=list(range(NCORES)),
                               trace=trace)
    outT = res.results[0]["outT"].astype(np.float32)
    for c in range(1, NCORES):
        outT += res.results[c]["outT"].astype(np.float32)
    out = outT.T.reshape(B, S, HID)
    return out, res


def kernel(**inputs):
    out, _ = _run(inputs, trace=False)
    return out

